# revision 7
# baseline (speedup 1.0000x reference)
"""AutoregressiveRAM kernel for trn2 (single NeuronCore, windowed one-hot design).

Per step (4096 sequential):
  bits [128,64] --(DVE mask-replicate)--> rhs bf16 [128,C]
  --(PE pack matmul, fixed weights 2^(k%16))--> PSUM word values
  --(ACT cast u16 + pos-word writes)--> swords [128, C+pos]
  --(DVE AND with per-(slot,color,word) masks + segmented reduce)--> sel2 = 2^r * tap_bit
  --(DVE mult by folded weights + reduce)--> addr f32 [128,64]
  --(ACT cast u16; DVE shift/AND)--> hi6, lo4
  --(DVE eq-onehot vs word iota * packed table + reduce)--> word u16
  --(DVE onehot(lo4) * 2^b + reduce -> pw; AND; >0)--> new bits
  --(SP DMA row out)

State-bit placement: offline greedy rainbow coloring (each neuron's 10 taps get
distinct colors) with capacity-aware placement into J windows of W words each.
"""
import sys
sys.path.insert(0, '/opt/trn_rl_repo')
import numpy as np

BITS = 8192
NB_T = 10
POS = 4
P = 128
S = 64          # neuron slots per partition
HI_W = 64       # table words per neuron (1024 bits / 16)

_CACHE = {}
TRACE = False
LAST = {}
N_STEPS = 4096


def _color_and_place(conn, J, W, seed=0):
    """Greedy rainbow coloring + slot placement.

    conn: [8192, 10] int32 tap bit-ids in [0, 8196).
    Returns None on failure, else dict with:
      color[8196], slot_k[8192], slot_n[8192] (hardware position of each state bit
      = of each neuron), wordcol[8196] (swords column of the bit's word),
      bitpos[8196] (bit position r in its word).
    Window j = swords cols [j*W, (j+1)*W). State windows use cols < C_s; the last
    `n_pos_colors` windows are pos windows (pos word replicated at their col 0).
    pack column c (c < C_s) maps to word (g=(c//64)%8, n=c%64): bits at
    partitions 16g..16g+15, column n.
    """
    rng = np.random.default_rng(seed)
    # adjacency: for each bit, the set of co-tapped bits
    nbrs = [[] for _ in range(BITS + POS)]
    for n in range(BITS):
        row = conn[n]
        for a in range(NB_T):
            for b in range(NB_T):
                if a != b and row[a] != row[b]:
                    nbrs[row[a]].append(row[b])
    nbrs = [np.unique(np.array(x, dtype=np.int32)) if x else np.zeros(0, np.int32)
            for x in nbrs]

    # pos bits: force into dedicated pos colors (last colors).
    # count max pos taps per neuron to know how many pos colors we need
    pos_taps = conn >= BITS
    max_pos = int(pos_taps.sum(1).max())
    n_pos_colors = max(1, max_pos)
    J_s = J - n_pos_colors
    C_s = J_s * W  # pack columns

    color = np.full(BITS + POS, -1, np.int32)
    # assign pos bits to pos colors: greedy 'no neuron has 2 taps in same pos color'
    # (pos conflict graph among the 4 pos bits)
    pos_conf = np.zeros((POS, POS), bool)
    for n in range(BITS):
        pp = conn[n][pos_taps[n]] - BITS
        for a in pp:
            for b in pp:
                if a != b:
                    pos_conf[a, b] = True
    for i in range(POS):
        used = {color[BITS + j] - J_s for j in range(POS)
                if color[BITS + j] >= 0 and pos_conf[i, j]}
        for c in range(n_pos_colors):
            if c not in used:
                color[BITS + i] = J_s + c
                break
        else:
            return None

    # slot pools per state color: list of (k, n, wordcol, bitpos)
    # word w_virtual = c in [0, C_s); (g, n) = ((c//64) % 8, c % 64); bits k=16g+r
    # bit slot (k, n) may be claimable by ANY color j whose window contains a col c
    # with (c//64)%8 == k//16 and c%64 == n.
    # For each (g, n) pair, list of covering colors:
    cover = [[] for _ in range(8 * 64)]
    col_of = {}  # (g, n, j) -> swords col
    for c in range(C_s):
        g, n = (c // 64) % 8, c % 64
        j = c // W
        cover[g * 64 + n].append(j)
        col_of[(g, n, j)] = c
    # free slots per color: dict color -> list of (k, n)
    free = [[] for _ in range(J_s)]
    for g in range(8):
        for n in range(64):
            for j in cover[g * 64 + n]:
                for r in range(16):
                    free[j].append((16 * g + r, n))
    # NOTE: a physical slot appears in multiple colors' free lists; track taken
    taken = np.zeros((P, 64), bool)
    free_ptr = [0] * J_s

    # order bits by degree descending
    deg = np.array([len(x) for x in nbrs[:BITS]])
    order = np.argsort(-deg, kind='stable')
    slot_k = np.full(BITS, -1, np.int32)
    slot_n = np.full(BITS, -1, np.int32)
    wordcol = np.full(BITS + POS, -1, np.int32)
    bitpos = np.full(BITS + POS, -1, np.int32)
    for i in range(POS):
        j = color[BITS + i]
        wordcol[BITS + i] = j * W  # pos word replicated at window start
        bitpos[BITS + i] = i

    cnt = np.zeros(J_s, np.int64)  # assigned per color
    for q in order:
        forb = set(color[nbrs[q]].tolist()) if len(nbrs[q]) else set()
        forb.discard(-1)
        best, best_load = -1, None
        for j in rng.permutation(J_s):
            if j in forb:
                continue
            # advance free_ptr past taken slots
            fl = free[j]
            while free_ptr[j] < len(fl) and taken[fl[free_ptr[j]][0], fl[free_ptr[j]][1]]:
                free_ptr[j] += 1
            if free_ptr[j] >= len(fl):
                continue
            if best == -1 or cnt[j] < best_load:
                best, best_load = j, cnt[j]
        if best == -1:
            return None
        j = best
        fl = free[j]
        k, n = fl[free_ptr[j]]
        free_ptr[j] += 1
        taken[k, n] = True
        color[q] = j
        slot_k[q] = k
        slot_n[q] = n
        g = k // 16
        wordcol[q] = col_of[(g, n, j)]
        bitpos[q] = k % 16
        cnt[j] += 1

    # any untaken slots remain free; any unassigned bits?
    if (color[:BITS] == -1).any():
        return None
    return dict(color=color, slot_k=slot_k, slot_n=slot_n,
                wordcol=wordcol, bitpos=bitpos, n_pos_colors=n_pos_colors,
                J_s=J_s, C_s=C_s)


def _build_constants(transition_memory, transition_connections, J, W, place):
    """All device constants, host-side numpy."""
    import ml_dtypes
    conn = transition_connections
    color = place['color']; slot_k = place['slot_k']; slot_n = place['slot_n']
    wordcol = place['wordcol']; bitpos = place['bitpos']
    C_s = -(-place['C_s'] // 64) * 64          # pack cols, rounded to 64
    CW = max(J * W, C_s)                       # swords total cols

    # neuron at hardware (p, s) = the bit placed there
    neuron_at = np.full((P, S), -1, np.int64)
    neuron_at[slot_k, slot_n] = np.arange(BITS)
    assert (neuron_at >= 0).all()

    # ANDMASK [P, S, J, W] u16 and POWC [P, S, J] f32
    ANDMASK = np.zeros((P, S, J, W), np.uint16)
    POWC = np.zeros((P, S, J), np.float64)
    for p in range(P):
        for s in range(S):
            n = neuron_at[p, s]
            for jt in range(NB_T):
                b = conn[n, jt]
                j = color[b]
                wc = wordcol[b]
                c_in_win = wc - j * W
                ANDMASK[p, s, j, c_in_win] |= np.uint16(1 << bitpos[b])
                POWC[p, s, j] += float(2 ** (NB_T - 1 - jt)) / float(2 ** bitpos[b])
    # POWC kept in f32 (duplicate-tap sums can exceed bf16 mantissa)
    assert np.array_equal(POWC.astype(np.float32).astype(np.float64), POWC), "POWC not exact in f32"

    # sanity: per (p,s,j) at most one distinct word (duplicates of same bit OK)
    # (rainbow property guarantees this)

    # MASKG bf16 [P, C_s]: gate [k//16 == (c//64)%8]
    cidx = np.arange(C_s)
    gofc = (cidx // 64) % 8
    MASKG = (np.arange(P)[:, None] // 16 == gofc[None, :]).astype(np.float32)

    # PACKW bf16 [P, P]: lhsT[k, m] = 2^(k%16)
    PACKW = np.tile((2.0 ** (np.arange(P) % 16))[:, None], (1, P)).astype(np.float32)

    # TBLT u16 [P, HI_W, S]: packed table words, transposed (w outer, s inner)
    T = transition_memory  # [8192, 1024] float 0/1
    Tb = (T > 0.5).astype(np.uint16).reshape(BITS, HI_W, 16)
    U16 = (Tb << np.arange(16, dtype=np.uint16)[None, None, :]).sum(-1).astype(np.uint16)  # [8192, 64]
    TBLT = np.zeros((P, HI_W, S), np.uint16)
    TBLT[:, :, :] = U16[neuron_at, :].transpose(0, 2, 1)  # [P, 64w, 64s]

    # IOTA_WT u16 [P, HI_W, S] = w ; IOTA16T u16 [P,16,S] = b ; POW2T u16 [P,16,S] = 1<<b
    IOTA_WT = np.tile(np.arange(HI_W, dtype=np.uint16)[None, :, None], (P, 1, S))
    IOTA16T = np.tile(np.arange(16, dtype=np.uint16)[None, :, None], (P, 1, S))
    POW2T = np.tile((np.uint16(1) << np.arange(16, dtype=np.uint16))[None, :, None], (P, 1, S))

    # POSW u16 [P, 4096]: pos word for steps t=1..4096 (index t-1)
    t = np.arange(1, 4097)
    posw = np.zeros(4096, np.uint16)
    for i in range(POS):
        posw |= (((t >> (3 - i)) & 1) << bitpos[BITS + i]).astype(np.uint16)
    POSW = np.tile(posw[None, :], (P, 1))

    pos_cols = sorted({int(wordcol[BITS + i]) for i in range(POS)})

    return dict(ANDMASK=ANDMASK, POWC=POWC.astype(np.float32), MASKG=MASKG,
                PACKW=PACKW, TBLT=TBLT, IOTA_WT=IOTA_WT, IOTA16T=IOTA16T,
                POW2T=POW2T, POSW=POSW, pos_cols=pos_cols, CW=CW, C_s=C_s,
                neuron_at=neuron_at)


def _f32c(u16arr):
    """pack a uint16 array (last dim even) into a float32 carrier"""
    a = np.ascontiguousarray(u16arr.reshape(P, -1))
    assert a.shape[1] % 2 == 0
    return a.view(np.float32)


def _bf16c(f32arr):
    import ml_dtypes
    a = np.ascontiguousarray(f32arr.reshape(P, -1)).astype(ml_dtypes.bfloat16)
    assert a.shape[1] % 2 == 0
    return a.view(np.uint16).view(np.float32)


def _build_program(J, W, C_s, CW, pos_cols, n_steps):
    from concourse import bacc, mybir
    SJ = S * J
    nc = bacc.Bacc('TRN2', target_bir_lowering=False, debug=False)
    dt = mybir.dt

    def param(name, cols, dtype=dt.float32):
        return nc.declare_dram_parameter(name, [P, cols], dtype, isOutput=False)

    x_andmask = param('x_andmask', (S * J * W) // 2)
    x_powc = param('x_powc', SJ)
    x_maskg = param('x_maskg', C_s // 2)
    x_packw = param('x_packw', P // 2)
    x_tblt = param('x_tblt', (HI_W * S) // 2)
    x_iotawt = param('x_iotawt', (HI_W * S) // 2)
    x_iota16 = param('x_iota16', (16 * S) // 2)
    x_pow2 = param('x_pow2', (16 * S) // 2)
    x_posw = param('x_posw', 4096 // 2)
    x_state0 = param('x_state0', S // 2)
    y = nc.declare_dram_parameter('y', [4096, BITS], dt.float32, isOutput=True)

    from contextlib import ExitStack
    es = ExitStack()
    block = es.enter_context(nc.Block())
    sb = lambda name, cols, dtype: es.enter_context(nc.sbuf_tensor(name, [P, cols], dtype))
    ANDMASK = sb('ANDMASK', S * J * W, dt.uint16)
    POWC = sb('POWC', SJ, dt.float32)
    MASKG = sb('MASKG', C_s, dt.bfloat16)
    PACKW = sb('PACKW', P, dt.bfloat16)
    TBLT = sb('TBLT', HI_W * S, dt.uint16)
    IOTA_WT = sb('IOTA_WT', HI_W * S, dt.uint16)
    IOTA16 = sb('IOTA16', 16 * S, dt.uint16)
    POW2 = sb('POW2', 16 * S, dt.uint16)
    POSW = sb('POSW', 4096, dt.uint16)
    swords = sb('swords', CW, dt.uint16)
    bitsrep = sb('bitsrep', C_s, dt.bfloat16)
    tmp1 = sb('tmp1', (S // 2) * J * W, dt.uint16)
    sel2 = sb('sel2', SJ, dt.uint16)
    addr = sb('addr', S, dt.float32)
    addrI = sb('addrI', S, dt.uint16)
    hi6 = sb('hi6', S, dt.uint16)
    lo4 = sb('lo4', S, dt.uint16)
    word2 = sb('word2', S, dt.uint16)
    pw = sb('pw', S, dt.uint16)
    tb = sb('tb', S, dt.uint16)
    bitu = sb('bitu', S, dt.uint16)
    bitf = sb('bitf', 2 * S, dt.float32)   # double-buffered f32 row
    psum = es.enter_context(nc.psum_tensor('psum', [P, C_s], dt.float32))

    s_in = es.enter_context(nc.semaphore('s_in'))
    s_rhs = es.enter_context(nc.semaphore('s_rhs'))
    s_pe = es.enter_context(nc.semaphore('s_pe'))
    s_sw = es.enter_context(nc.semaphore('s_sw'))
    s_addr = es.enter_context(nc.semaphore('s_addr'))
    s_ai = es.enter_context(nc.semaphore('s_ai'))
    s_bits = es.enter_context(nc.semaphore('s_bits'))
    s_bf = es.enter_context(nc.semaphore('s_bf'))
    s_dma = es.enter_context(nc.semaphore('s_dma'))

    N_IN_DMAS = 10

    AND = mybir.AluOpType.bitwise_and
    MULT = mybir.AluOpType.mult
    ADD = mybir.AluOpType.add
    EQ = mybir.AluOpType.is_equal
    X = mybir.AxisListType.X

    @block.sync
    def _(sync):
        for name, t_, src in [('ANDMASK', ANDMASK, x_andmask), ('POWC', POWC, x_powc),
                              ('MASKG', MASKG, x_maskg), ('PACKW', PACKW, x_packw),
                              ('TBLT', TBLT, x_tblt), ('IOTA_WT', IOTA_WT, x_iotawt),
                              ('IOTA16', IOTA16, x_iota16), ('POW2', POW2, x_pow2),
                              ('POSW', POSW, x_posw), ('bitu', bitu, x_state0)]:
            dtt = t_.tensor.dtype if hasattr(t_, 'tensor') else None
            sync.dma_start(out=t_[:], in_=src[:].bitcast(t_.dtype)).then_inc(s_in, 16)
        # per-iteration output DMA
        import concourse.bass as bass_mod
        with sync.Fori(0, n_steps) as row:
            sync.wait_ge(s_bf, row + 1)
            off = sync.scalar_reg_alu(mybir.AluOpType.bitwise_and, row, 1)
            sync.dma_start(
                out=y[bass_mod.ds(row, 1), :],
                in_=bitf[:].rearrange('p (b s) -> p b s', b=2)[:, bass_mod.ds(off, 1), :],
            ).then_inc(s_dma, 16)

    @block.tensor
    def _(tensor):
        tensor.wait_ge(s_in, 16 * N_IN_DMAS)
        with tensor.Fori(1, n_steps + 1) as it:
            tensor.wait_ge(s_rhs, it)
            # wait ACT consumed previous psum
            tensor.wait_ge(s_sw, it - 1)
            tensor.matmul(psum[:, 0:512], PACKW[:], bitsrep[:, 0:512], start=True, stop=True)
            tensor.matmul(psum[:, 512:C_s], PACKW[:], bitsrep[:, 512:C_s],
                          start=True, stop=True).then_inc(s_pe, 1)

    @block.scalar
    def _(scalar):
        scalar.wait_ge(s_in, 16 * N_IN_DMAS)
        import concourse.bass as bass_mod
        with scalar.Fori(1, n_steps + 1) as it:
            scalar.wait_ge(s_pe, it)
            PC0 = min(pos_cols)
            scalar.copy(out=swords[:, 0:PC0], in_=psum[:, 0:PC0])
            tm1 = scalar.snap(it - 1, min_val=0, max_val=n_steps - 1)
            for i_pc, pc in enumerate(pos_cols):
                ins_pc = scalar.copy(out=swords[:, pc:pc + 1],
                                     in_=POSW[:, bass_mod.ds(tm1, 1)])
                if i_pc == len(pos_cols) - 1:
                    ins_pc.then_inc(s_sw, 1)
            # addr cast
            scalar.wait_ge(s_addr, it)
            scalar.copy(out=addrI[:], in_=addr[:]).then_inc(s_ai, 1)
            # bit outputs
            scalar.wait_ge(s_bits, it)
            dtgt = scalar.scalar_reg_alu(mybir.AluOpType.mult, scalar.snap(it - 1, min_val=0, max_val=n_steps), 16)
            scalar.wait_ge(s_dma, dtgt)
            off = scalar.scalar_reg_alu(mybir.AluOpType.bitwise_and, tm1, 1)
            scalar.copy(out=bitf[:].rearrange('p (b s) -> p b s', b=2)[:, bass_mod.ds(off, 1), :],
                        in_=bitu[:]).then_inc(s_bf, 1)

    @block.vector
    def _(vector):
        _lp = nc.allow_low_precision(reason='u16 one-hot reductions, exact by construction')
        _lp.__enter__()
        vector.wait_ge(s_in, 16 * N_IN_DMAS)
        vector.memset(swords[:], 0)
        # prologue: bitsrep for it=1 from state0 (in bitu)
        vector.tensor_tensor(bitsrep[:].rearrange('p (r n) -> p r n', n=S),
                             bitu[:].unsqueeze(1).broadcast_to([P, C_s // S, S]),
                             MASKG[:].rearrange('p (r n) -> p r n', n=S),
                             MULT).then_inc(s_rhs, 1)
        HS = S // 2
        addrt = tmp1[:].bitcast(dt.float32)[:, 0:SJ]
        onehot = tmp1[:, 0:HI_W * S]
        wtmp = tmp1[:, HI_W * S:2 * HI_W * S]
        oh16 = tmp1[:, 2 * HI_W * S:2 * HI_W * S + 16 * S]
        ptmp = tmp1[:, 2 * HI_W * S + 16 * S:2 * HI_W * S + 32 * S]
        with vector.Fori(1, n_steps + 1) as it:
            vector.wait_ge(s_sw, it)
            # G1: AND + segmented reduce, two s-halves sharing tmp1
            for h in range(2):
                vector.tensor_tensor(
                    tmp1[:].rearrange('p (s j c) -> p s j c', s=HS, j=J),
                    swords[:, 0:J * W].rearrange('p (j c) -> p j c', j=J).unsqueeze(1).broadcast_to([P, HS, J, W]),
                    ANDMASK[:].rearrange('p (s j c) -> p s j c', s=S, j=J)[:, h * HS:(h + 1) * HS],
                    AND)
                vector.drain()
                vector.tensor_reduce(sel2[:].rearrange('p (s j) -> p s j', s=S)[:, h * HS:(h + 1) * HS],
                                     tmp1[:].rearrange('p (s j c) -> p s j c', s=HS, j=J),
                                     axis=X, op=ADD)
                vector.drain()
            # addr = sum_j sel2 * POWC
            vector.drain()
            vector.tensor_tensor(addrt.rearrange('p (s j) -> p s j', s=S),
                                 sel2[:].rearrange('p (s j) -> p s j', s=S),
                                 POWC[:].rearrange('p (s j) -> p s j', s=S),
                                 MULT)
            vector.drain()
            vector.tensor_reduce(addr[:].rearrange('p s -> p s'),
                                 addrt.rearrange('p (s j) -> p s j', s=S),
                                 axis=X, op=ADD).then_inc(s_addr, 1)
            vector.drain()
            vector.wait_ge(s_ai, it)
            vector.tensor_scalar(hi6[:], addrI[:], 4, None, mybir.AluOpType.logical_shift_right)
            vector.tensor_scalar(lo4[:], addrI[:], 15, None, AND)
            vector.drain()
            # G2: onehot over words (transposed: w outer, s inner)
            vector.tensor_tensor(onehot.rearrange('p (w s) -> p w s', w=HI_W),
                                 hi6[:].unsqueeze(1).broadcast_to([P, HI_W, S]),
                                 IOTA_WT[:].rearrange('p (w s) -> p w s', w=HI_W),
                                 EQ)
            vector.drain()
            vector.tensor_tensor(wtmp.rearrange('p (w s) -> p w s', w=HI_W),
                                 onehot.rearrange('p (w s) -> p w s', w=HI_W),
                                 TBLT[:].rearrange('p (w s) -> p w s', w=HI_W),
                                 MULT)
            vector.drain()
            vector.tensor_reduce(word2[:].rearrange('p s -> p s'),
                                 wtmp.rearrange('p (w s) -> p w s', w=HI_W).transpose([0, 2, 1]),
                                 axis=X, op=ADD)
            vector.drain()
            # pw = 1 << lo4 via onehot16
            vector.tensor_tensor(oh16.rearrange('p (b s) -> p b s', b=16),
                                 lo4[:].unsqueeze(1).broadcast_to([P, 16, S]),
                                 IOTA16[:].rearrange('p (b s) -> p b s', b=16),
                                 EQ)
            vector.drain()
            vector.tensor_tensor(ptmp.rearrange('p (b s) -> p b s', b=16),
                                 oh16.rearrange('p (b s) -> p b s', b=16),
                                 POW2[:].rearrange('p (b s) -> p b s', b=16),
                                 MULT)
            vector.drain()
            vector.tensor_reduce(pw[:].rearrange('p s -> p s'),
                                 ptmp.rearrange('p (b s) -> p b s', b=16).transpose([0, 2, 1]),
                                 axis=X, op=ADD)
            vector.drain()
            vector.tensor_tensor(tb[:], word2[:], pw[:], AND)
            vector.drain()
            vector.tensor_scalar(bitu[:], tb[:], 0, None, mybir.AluOpType.is_gt).then_inc(s_bits, 1)
            # next-iteration rhs
            vector.drain()
            vector.tensor_tensor(bitsrep[:].rearrange('p (r n) -> p r n', n=S),
                                 bitu[:].unsqueeze(1).broadcast_to([P, C_s // S, S]),
                                 MASKG[:].rearrange('p (r n) -> p r n', n=S),
                                 MULT).then_inc(s_rhs, 1)
        _lp.__exit__(None, None, None)

    es.close()
    nc.finalize()
    return nc


def kernel(transition_memory, initial_memory, transition_connections,
           initial_connections, length):
    from concourse.bass_utils import run_bass_kernel_spmd
    length = int(length)
    conn = np.asarray(transition_connections)
    key = ('prog', N_STEPS)
    if key not in _CACHE:
        # coloring
        place = None
        for (J, W) in [(54, 13), (62, 12), (70, 11), (80, 10)]:
            place = _color_and_place(conn, J, W)
            if place is not None:
                break
        assert place is not None, 'coloring failed'
        consts = _build_constants(np.asarray(transition_memory), conn, J, W, place)
        prog = _build_program(J, W, consts['C_s'], consts['CW'], consts['pos_cols'], N_STEPS)
        _CACHE[key] = (J, W, place, consts, prog)
    J, W, place, consts, prog = _CACHE[key]

    # step 0 host-side: pos bits of step 0 are all zero -> addr0 per neuron
    ic = np.asarray(initial_connections)
    im = np.asarray(initial_memory)
    pos0 = np.zeros(POS, np.int64)
    sel0 = pos0[ic]                      # [8192, 4]
    pw0 = 2 ** np.arange(3, -1, -1)
    addr0 = (sel0 * pw0).sum(1)
    out0 = im[np.arange(BITS), addr0].astype(np.float32)
    state0_bits = (out0 > 0.5).astype(np.uint16)

    # state0 at hardware positions
    neuron_at = consts['neuron_at']
    st0 = state0_bits[neuron_at]        # [P, S] u16

    ins = {
        'x_andmask': _f32c(consts['ANDMASK']),
        'x_powc': np.ascontiguousarray(consts['POWC'].reshape(P, -1)),
        'x_maskg': _bf16c(consts['MASKG']),
        'x_packw': _bf16c(consts['PACKW']),
        'x_tblt': _f32c(consts['TBLT']),
        'x_iotawt': _f32c(consts['IOTA_WT']),
        'x_iota16': _f32c(consts['IOTA16T']),
        'x_pow2': _f32c(consts['POW2T']),
        'x_posw': _f32c(consts['POSW']),
        'x_state0': _f32c(st0),
    }
    res = run_bass_kernel_spmd(prog, [ins], core_ids=[0])
    LAST['res'] = res
    LAST['ins'] = ins
    LAST['prog'] = prog
    dev = res.results[0]['y']           # [4096, 8192]: row r = step r+1, cols = (p*64+s)

    out = np.empty((length, BITS), np.float32)
    out[0] = out0
    # unpermute: neuron q sits at column (slot_k*64 + slot_n)
    colidx = place['slot_k'] * 64 + place['slot_n']
    out[1:length] = dev[0:length - 1][:, colidx]
    return out

